# revision 37
# baseline (speedup 1.0000x reference)
"""Sequence-parallel fused LayerNorm + QKV-projection + attention for TRN2.

x [8192,10] f32 -> LN -> h @ W.T -> q,k,v -> out = softmax(q k^T) v [8192,11].
The 8192 query rows are sharded across 8 NeuronCores (1024 each); every core
computes k/v for all keys itself (projection is tiny, no collectives).

v2 design (from the ~86us v1): the main loop is bound by the PSUM->engine
read port (32b/cycle/lane on DVE and ACT; GPSIMD and DMA cannot touch PSUM),
so everything else is folded away:
- The Schraudolph exp transform (i16 = round(sim*1024*log2e + 15*1024 - C),
  bitcast to fp16) is fused INTO the qk matmul: q is pre-scaled by
  A10*SCALE on the host and a 12th constant feature (k=1, q=b10c) adds the
  offset. Both DVE and ACT then drain sim tiles with plain f32->i16
  round-convert copies (verified round-to-nearest on HW), costing exactly
  the PSUM port bound. ACT takes 8/15 tiles, DVE 7/15 (rate-balanced).
- v row-major obtained by tiled DMA xbar transposes ([32,1024] fp16 ->
  [128,8,32]) ALL on the sync queue (v1 put 2 on the scalar queue, which
  now must spend every cycle draining PSUM); explicit sync deps on both
  sides as before.
- keys packed 4 tiles/column-group at partition bases {0,32,64,96}; qk h0
  reads kvT on band b, qk h1 reads a 64-partition-rotated copy (kvT2) on
  band (b+2)%4, so both halves run concurrently on distinct (row-band,
  PSUM-bank) pairs. Concurrent row-tiled MMs into the SAME PSUM bank fault.
- av halves spread over all 4 col groups via (t + 2h) % 4; merged in the
  epilogue after a transpose, divided by the fused denominator column.
- single shared PSUM pool (3x [128,1024] f32 sim slots + out_big) from t=0
  so prologue transposes interleave with the first qk tiles; consts are
  uploaded pre-cast fp16; x arrives in 4 chunks on 3 DMA queues.
"""

import ml_dtypes
import numpy as np

import concourse.bass as bass
import concourse.bacc as bacc
from concourse import mybir
from concourse.tile import TileContext
from concourse.tile_rust import add_dep_helper
from concourse.bass_utils import run_bass_kernel_spmd

F32 = mybir.dt.float32
BF16 = mybir.dt.bfloat16
I16 = mybir.dt.int16

N = 8192
NCORES = 8
NQ = N // NCORES
P = 128
R = N // P
RQ = NQ // P
D = 10
DA = D + 1
KO = 11
KF = 12           # k features incl. the b10c-partner constant
VA = 12           # v features incl. the denominator-ones column
V_OFF = 12        # v offset inside each 32-wide kv band
VSS = 32          # vS slot stride (full transposed kv block)
NM = R // 4
NC = NM * P
NJ = 64
EPS = 1e-5
SCALE = D ** -0.5

LOG2E = 1.4426950408889634
A10 = 128.0 * LOG2E
C_TUNED = 7.625

# fp16 consts: identH | wkvB | wq (12)
CWH = P + P + KF


def _build_nc():
    nc = bacc.Bacc(None, target_bir_lowering=False)

    x_d = nc.dram_tensor("x", [N, D], F32, kind="ExternalInput")
    xq_d = nc.dram_tensor("xq", [NQ, D], F32, kind="ExternalInput")
    csth_d = nc.dram_tensor("csth", [P, CWH], BF16, kind="ExternalInput")
    cst32_d = nc.dram_tensor("cst32", [P, VA], BF16, kind="ExternalInput")
    y_d = nc.dram_tensor("y", [NQ, KO], F32, kind="ExternalOutput")

    with TileContext(nc) as tc:
        with (
            tc.tile_pool(name="const", bufs=1) as constp,
            tc.tile_pool(name="big", bufs=1) as bigp,
        ):
            csth = constp.tile([P, CWH], BF16)
            nc.sync.dma_start(out=csth, in_=csth_d[:])
            identH = csth[:, 0:P]
            wkvh = csth[:, P : 2 * P]
            wqh = csth[0:DA, 2 * P : 2 * P + KF]

            # 4-group merge matrix for the epilogue matmul-merge
            mrg = constp.tile([P, VA], BF16)

            xnT = bigp.tile([P, NC], BF16)
            kvT = bigp.tile([P, NC], BF16)
            kvT2 = bigp.tile([P, NC], BF16)   # kvT rotated 64 partitions
            qR = bigp.tile([P, NQ], BF16)
            vS = bigp.tile([P, NJ * VSS], BF16)  # transposed kv blocks
            xqT = bigp.tile([DA, NQ], BF16)

            with tc.tile_pool(name="work", bufs=1) as workp:
                pstp_cm = tc.tile_pool(name="pst", bufs=2, space="PSUM")
                pstp = pstp_cm.__enter__()

                def mk_pt():
                    return pstp.tile([P, 512], BF16, name="ptx", tag="ps")

                def mk_pk():
                    return pstp.tile([P, 512], F32, name="pk", tag="ps")

                def ln_stats(xr, nrows_p, name, sq_on_act=False):
                    """stats chain -> (mu, tenvar=10*var)."""
                    sq = workp.tile([P, nrows_p, D], F32, name=f"sq_{name}")
                    if sq_on_act:
                        nc.scalar.activation(
                            out=sq, in_=xr,
                            func=mybir.ActivationFunctionType.Square,
                            bias=0.0, scale=1.0)
                    else:
                        nc.vector.tensor_mul(sq, xr, xr)
                    s1 = workp.tile([P, nrows_p], F32, name=f"s1_{name}")
                    nc.vector.reduce_sum(out=s1, in_=xr, axis=mybir.AxisListType.X)
                    tv = workp.tile([P, nrows_p], F32, name=f"tv_{name}")
                    nc.vector.reduce_sum(out=tv, in_=sq, axis=mybir.AxisListType.X)
                    mu = workp.tile([P, nrows_p], F32, name=f"mu_{name}")
                    nc.vector.tensor_scalar_mul(mu, s1, 1.0 / D)
                    musq = workp.tile([P, nrows_p], F32, name=f"musq_{name}")
                    nc.vector.tensor_mul(musq, mu, s1)
                    nc.vector.tensor_sub(tv, tv, musq)
                    return mu, tv

                def rsig_of(tv, nrows_p, name):
                    """1/sqrt(tenvar/10 + eps): ACT Sqrt + DVE reciprocal."""
                    sg = workp.tile([P, nrows_p], F32, name=f"sg_{name}")
                    nc.scalar.activation(
                        out=sg, in_=tv,
                        func=mybir.ActivationFunctionType.Sqrt,
                        bias=eps, scale=1.0 / D)
                    rs = workp.tile([P, nrows_p], F32, name=f"rs_{name}")
                    nc.vector.reciprocal(rs, sg)
                    return rs

                def ln_finish(xr, xa_slice, mu, rsig, nrows_p, sub,
                              eng=None):
                    e = eng or nc.vector
                    for h0 in range(0, nrows_p, sub):
                        h1 = min(h0 + sub, nrows_p)
                        nh = h1 - h0
                        e.tensor_sub(
                            xa_slice[:, h0:h1, 0:D], xr[:, h0:h1, :],
                            mu[:, h0:h1].broadcast_to([P, nh, D]),
                        )
                        e.tensor_mul(
                            xa_slice[:, h0:h1, 0:D], xa_slice[:, h0:h1, 0:D],
                            rsig[:, h0:h1].broadcast_to([P, nh, D]),
                        )
                        e.memset(xa_slice[:, h0:h1, D : D + 1], 1.0)

                # ---------- DMAs (before any ACT op: the sqrt table load
                # otherwise blocks the scalar DMA queue for ~2.6us) ----------
                xq_r = workp.tile([P, RQ, D], F32, name="xr_q")
                nc.scalar.dma_start(
                    out=xq_r, in_=xq_d.rearrange("(p r) c -> p r c", p=P)
                )
                x_r = workp.tile([P, R, D], F32, name="xr_x")
                x_src = x_d.rearrange("(p r) c -> p r c", p=P)
                nc.sync.dma_start(out=x_r[:, 0:16, :], in_=x_src[:, 0:16, :])
                nc.gpsimd.dma_start(out=x_r[:, 16:32, :], in_=x_src[:, 16:32, :])
                nc.scalar.dma_start(out=x_r[:, 32:48, :], in_=x_src[:, 32:48, :])
                nc.scalar.dma_start(out=x_r[:, 48:64, :], in_=x_src[:, 48:64, :])
                # merge matrix only needed by the epilogue
                nc.scalar.dma_start(out=mrg, in_=cst32_d[:])

                eps = constp.tile([P, 1], F32)
                nc.vector.memset(eps, EPS)
                # dummy Sqrt pulls the sqrt table set load under the DMAs
                scr = constp.tile([P, 2], F32)
                nc.scalar.activation(
                    out=scr[:, 0:1], in_=eps,
                    func=mybir.ActivationFunctionType.Sqrt, bias=0.0, scale=1.0)
                # pad columns of the kv activations, all slices in one shot
                xa = workp.tile([P, R, 32], BF16, name="xa_x")
                nc.gpsimd.memset(xa[:, :, DA:32], 0.0)

                # ---------- x slice 0 first: its data arrives first and
                # it gates the first qk quad's kvT chunk ----------
                x_stats = {}

                def x_stats_for(s):
                    xrs = x_r[:, 16 * s : 16 * s + 16, :]
                    mu, tv = ln_stats(xrs, 16, f"x{s}", sq_on_act=True)
                    rs = rsig_of(tv, 16, f"x{s}")
                    x_stats[s] = (mu, rs)

                def x_stats_for23():
                    mu, tv = ln_stats(x_r[:, 32:64, :], 32, "x23",
                                      sq_on_act=True)
                    rs = rsig_of(tv, 32, "x23")
                    x_stats[2] = (mu[:, 0:16], rs[:, 0:16])
                    x_stats[3] = (mu[:, 16:32], rs[:, 16:32])

                qrdma = [None] * 4

                def q_side():
                    q_mu, q_tv = ln_stats(xq_r, RQ, "q")
                    q_rs = rsig_of(q_tv, RQ, "q")
                    xqa = workp.tile([P, RQ, DA], BF16, name="xa_q")
                    ln_finish(xq_r, xqa, q_mu, q_rs, RQ, RQ, eng=nc.gpsimd)
                    for g in range(RQ // 4):
                        pt = mk_pt()
                        for k4 in range(4):
                            r = g * 4 + k4
                            nc.tensor.transpose(
                                pt[0:DA, k4 * P : (k4 + 1) * P], xqa[:, r, :],
                                identH,
                            )
                        nc.vector.tensor_copy(
                            xqT[:, g * 512 : (g + 1) * 512], pt[0:DA, :]
                        )
                    pqcopies = []
                    for t in range(NQ // 512):
                        pq = mk_pk()
                        nc.tensor.matmul(
                            pq[0:KF, :], wqh, xqT[:, t * 512 : (t + 1) * 512],
                            start=True, stop=True,
                        )
                        if t % 2 == 0:
                            pqcopies.append(nc.vector.tensor_copy(
                                qR[0:KF, t * 512 : (t + 1) * 512], pq[0:KF, :]))
                        else:
                            pqcopies.append(nc.scalar.copy(
                                qR[0:KF, t * 512 : (t + 1) * 512], pq[0:KF, :]))
                    for bi, rp in enumerate((32, 64, 96)):
                        qrdma[bi + 1] = nc.sync.dma_start(
                            out=qR[rp : rp + KF, :], in_=qR[0:KF, :])
                        for cp_ in pqcopies:
                            add_dep_helper(qrdma[bi + 1].ins, cp_.ins, sync=True,
                                           reason="qR replicate after pq copies")

                # explicit sync deps: the DMA-written kvT2/vS consumers race
                # without them (dep tracking misses the strided DMA outputs)
                k2dma = [[None] * 4 for _ in range(4)]
                vdma = [[None] * 4 for _ in range(2)]
                kvcopy = [None] * 4

                # ---------- per-slice normalize/transpose/proj/kvT/v ------
                def x_slice(s):
                    r0 = 16 * s
                    mu, rs = x_stats[s]
                    xah = xa[:, r0 : r0 + 16, :]
                    # NOTE: ln_finish must NOT share the gpsimd queue with
                    # the k2dma descriptor gens -- the scheduler interleaves
                    # them into a cross-engine FIFO cycle (14us stall).
                    ln_finish(x_r[:, r0 : r0 + 16, :], xah, mu, rs, 16, 16,
                              eng=nc.vector)
                    ch = s
                    pt = mk_pt()
                    for mi in range(4):
                        m = ch * 4 + mi
                        nc.tensor.transpose(
                            pt[:, mi * P : (mi + 1) * P],
                            xa[:, m * 4 : m * 4 + 4, :], identH,
                        )
                    dst = xnT[:, ch * 512 : (ch + 1) * 512]
                    if ch % 2 == 0:
                        nc.vector.tensor_copy(dst, pt)
                    else:
                        nc.scalar.copy(dst, pt)
                    pk = mk_pk()
                    nc.tensor.matmul(
                        pk, wkvh, xnT[:, ch * 512 : (ch + 1) * 512],
                        start=True, stop=True,
                    )
                    dstk = kvT[:, ch * 512 : (ch + 1) * 512]
                    if ch % 2 == 0:
                        kvcopy[ch] = nc.scalar.copy(dstk, pk)
                    else:
                        kvcopy[ch] = nc.vector.tensor_copy(dstk, pk)
                    # rotate the k rows by 64 partitions so qk h1 can use
                    # row band (b+2)%4: both qk MMs of a tile run
                    # concurrently. gpsimd DMA queue is idle here.
                    cs = slice(ch * 512, (ch + 1) * 512)
                    for b in range(4):
                        bb = (b + 2) % 4
                        k2dma[ch][bb] = nc.gpsimd.dma_start(
                            out=kvT2[32 * bb : 32 * bb + KF, cs],
                            in_=kvT[32 * b : 32 * b + KF, cs])
                        add_dep_helper(k2dma[ch][bb].ins, kvcopy[ch].ins,
                                       sync=True, reason="kvT2 dma after copy")

                def v_half(h):
                    # v row-major: one tiled DMA xbar transpose per base --
                    # in [32, 1024] -> out [128, 8, 32] (8 kv blocks)
                    vS_r4 = vS.rearrange("p (m b c) -> p m b c", b=4, c=VSS)
                    for b in range(4):
                        vdma[h][b] = nc.sync.dma_start_transpose(
                            out=vS_r4[:, h * 8 : (h + 1) * 8, b, :],
                            in_=kvT[32 * b : 32 * b + 32,
                                    h * 1024 : (h + 1) * 1024],
                        )
                        add_dep_helper(vdma[h][b].ins, kvcopy[2 * h].ins,
                                       sync=True, reason="v dma after kv copies")
                        add_dep_helper(vdma[h][b].ins, kvcopy[2 * h + 1].ins,
                                       sync=True, reason="v dma after kv copies")

                x_stats_for(0)
                x_slice(0)
                q_side()
                x_stats_for(1)
                x_slice(1)
                v_half(0)
                x_stats_for23()
                x_slice(2)
                x_slice(3)
                v_half(1)
                pstp_cm.__exit__(None, None, None)

                # ---------- attention main loop ----------
                outp_cm = tc.tile_pool(name="outp", bufs=1, space="PSUM")
                outp = outp_cm.__enter__()
                out_big = outp.tile([P, NQ], F32)
                simp_cm = tc.tile_pool(name="simp", bufs=3, space="PSUM")
                simp = simp_cm.__enter__()

                def mk_sim():
                    return simp.tile([P, NQ], F32, name="sim")

                with tc.tile_pool(name="expp", bufs=12) as expp:

                    def emit_av(t, et):
                        vj = vS[:, t * VSS + V_OFF : t * VSS + V_OFF + VA]
                        vd = vdma[t // 32][t % 4]
                        for hh in range(NQ // 512):
                            cp = ((t + 2 * hh) % 4) * 32
                            mm = nc.tensor.matmul(
                                out_big[cp : cp + VA, hh * 512 : (hh + 1) * 512],
                                vj, et[:, hh * 512 : (hh + 1) * 512],
                                start=(t < 2), stop=(t >= NJ - 2),
                                tile_position=(0, cp),
                            )
                            if ("v", t // 32, t % 4) not in dep_done:
                                dep_done.add(("v", t // 32, t % 4))
                                add_dep_helper(mm.ins, vd.ins, sync=True,
                                               reason="av after v xbar dma")

                    dep_done = set()

                    def emit_qk(t):
                        m, b = t // 4, t % 4
                        sim = mk_sim()
                        for hh in range(NQ // 512):
                            # h1 reads the rotated copy on band (b+2)%4
                            bb = (b + 2 * hh) % 4
                            rp = bb * 32
                            src = kvT if hh == 0 else kvT2
                            qk = nc.tensor.matmul(
                                sim[:, hh * 512 : (hh + 1) * 512],
                                src[rp : rp + KF, m * P : (m + 1) * P],
                                qR[rp : rp + KF, hh * 512 : (hh + 1) * 512],
                                start=True, stop=True,
                                tile_position=(rp, 0),
                            )
                            if bb > 0 and ("qr", bb) not in dep_done:
                                dep_done.add(("qr", bb))
                                add_dep_helper(
                                    qk.ins, qrdma[bb].ins,
                                    sync=True, reason="qk after qR replicate")
                            if hh == 1 and ("k2", t // 16, bb) not in dep_done:
                                dep_done.add(("k2", t // 16, bb))
                                add_dep_helper(
                                    qk.ins, k2dma[t // 16][bb].ins,
                                    sync=True, reason="qk h1 after kvT2 dma")
                        return sim

                    def emit_drain(t, sim):
                        et = expp.tile([P, NQ], BF16, name="et")
                        # fused Schraudolph: sim already = A10*qk + b10c;
                        # f32->i16 convert-copy rounds to nearest (HW-checked).
                        # ACT is faster per tile: 5:3 ACT-heavy pattern
                        if t % 8 in (0, 3, 5):
                            nc.vector.tensor_copy(et[:].bitcast(I16), sim)
                        else:
                            nc.scalar.copy(et[:].bitcast(I16), sim)
                        return et

                    # batches of 2 tiles: the 4 qk MMs cover all 4 row
                    # bands and sit adjacent in the PE FIFO (concurrent);
                    # same for the 4 av MMs over all 4 col groups
                    pend = []
                    for g in range(NJ // 2):
                        ts = (2 * g, 2 * g + 1)
                        sims = [emit_qk(t) for t in ts]
                        ets = [emit_drain(t, s) for t, s in zip(ts, sims)]
                        pend.append((ts, ets))
                        if len(pend) > 2:
                            ts_, ets_ = pend.pop(0)
                            for t_, e_ in zip(ts_, ets_):
                                emit_av(t_, e_)
                    for ts_, ets_ in pend:
                        for t_, e_ in zip(ts_, ets_):
                            emit_av(t_, e_)

                # ---------- epilogue ----------
                simp_cm.__exit__(None, None, None)
                with tc.tile_pool(name="ep", bufs=1) as epp, \
                     tc.tile_pool(name="epps", bufs=1, space="PSUM") as eppsp:
                    MW = 108
                    oS = epp.tile([P, NQ], BF16)
                    nc.vector.tensor_copy(oS[0:MW, 0:512], out_big[0:MW, 0:512])
                    nc.scalar.copy(oS[0:MW, 512:1024], out_big[0:MW, 512:1024])
                    # transpose + 4-group merge in one matmul per q-chunk:
                    # out[q, f] = sum_p oS[p, q] * mrg[p, f]
                    pm = eppsp.tile([P, RQ * VA], F32, name="pm")
                    for ch in range(RQ):
                        nc.tensor.matmul(
                            pm[:, ch * VA : (ch + 1) * VA],
                            oS[0:MW, ch * P : (ch + 1) * P],
                            mrg[0:MW, :],
                            start=True, stop=True,
                        )
                    oM = epp.tile([P, RQ, VA], F32)
                    nc.vector.tensor_copy(
                        oM, pm.rearrange("p (t c) -> p t c", c=VA))
                    rec = epp.tile([P, RQ], F32)
                    nc.vector.reciprocal(rec, oM[:, :, KO])
                    oF = epp.tile([P, RQ, KO], F32)
                    nc.vector.tensor_mul(
                        oF, oM[:, :, 0:KO], rec.broadcast_to([P, RQ, KO])
                    )
                    nc.scalar.dma_start(
                        out=y_d.rearrange("(p t) c -> p t c", p=P), in_=oF
                    )
                outp_cm.__exit__(None, None, None)
    nc.compile()
    return nc


_NC_CACHE = {}


def _get_nc():
    if "nc" not in _NC_CACHE:
        _NC_CACHE["nc"] = _build_nc()
    return _NC_CACHE["nc"]


def _host_prep(x, gamma, beta, W):
    x = np.asarray(x, np.float32)
    gamma = np.asarray(gamma, np.float32)
    beta = np.asarray(beta, np.float32)
    W = np.asarray(W, np.float32)
    Wg = W * gamma[None, :]
    b0 = W @ beta
    Wq, Wk, Wv = Wg[0:KO], Wg[KO : 2 * KO], Wg[2 * KO : 3 * KO]
    bq, bk, bv = b0[0:KO], b0[KO : 2 * KO], b0[2 * KO : 3 * KO]

    b10c = 127.0 * 128.0 - C_TUNED
    a10s = A10 * SCALE

    # block-diagonal kv projection, per 32-band: 11 k features, then the
    # constant-1 partner of the b10c offset, then 11 v features + the
    # denominator-ones column at V_OFF..V_OFF+VA
    wkvB = np.zeros((P, P), np.float32)
    for b in range(4):
        o = 32 * b
        wkvB[o : o + D, o : o + KO] = Wk.T
        wkvB[o + D, o : o + KO] = bk
        wkvB[o + D, o + KO] = 1.0
        wkvB[o : o + D, o + V_OFF : o + V_OFF + KO] = Wv.T
        wkvB[o + D, o + V_OFF : o + V_OFF + KO] = bv
        wkvB[o + D, o + V_OFF + KO] = 1.0

    # q projection pre-scaled by A10*SCALE; col 11 emits the b10c constant
    wq_a = np.zeros((DA, KF), np.float32)
    wq_a[0:D, 0:KO] = Wq.T * a10s
    wq_a[D, 0:KO] = bq * a10s
    wq_a[D, KO] = b10c

    csth = np.zeros((P, CWH), np.float32)
    csth[:, 0:P] = np.eye(P)
    csth[:, P : 2 * P] = wkvB
    csth[0:DA, 2 * P : 2 * P + KF] = wq_a
    csth = csth.astype(ml_dtypes.bfloat16)
    # merge matrix: sum the 4 column-group partials during the epilogue
    # transpose-matmul; out[q, f] = sum_g out_big[32g + f, q]
    cst32 = np.zeros((P, VA), np.float32)
    for g in range(4):
        for f in range(VA):
            cst32[32 * g + f, f] = 1.0
    cst32 = cst32.astype(ml_dtypes.bfloat16)
    return x, csth, cst32


def _run(x, gamma, beta, W, **spmd_kwargs):
    nc = _get_nc()
    x, csth, cst32 = _host_prep(x, gamma, beta, W)
    in_maps = []
    for c in range(NCORES):
        in_maps.append({
            "x": x,
            "xq": np.ascontiguousarray(x[c * NQ : (c + 1) * NQ]),
            "csth": csth,
            "cst32": cst32,
        })
    res = run_bass_kernel_spmd(
        nc, in_maps, core_ids=list(range(NCORES)), **spmd_kwargs
    )
    out = np.concatenate([res.results[c]["y"] for c in range(NCORES)], axis=0)
    return out, res


def kernel(x, gamma, beta, W):
    out, _ = _run(x, gamma, beta, W)
    return out


# revision 38
# speedup vs baseline: 1.0054x; 1.0054x over previous
"""Sequence-parallel fused LayerNorm + QKV-projection + attention for TRN2.

x [8192,10] f32 -> LN -> h @ W.T -> q,k,v -> out = softmax(q k^T) v [8192,11].
The 8192 query rows are sharded across 8 NeuronCores (1024 each); every core
computes k/v for all keys itself (projection is tiny, no collectives).

v2 design (from the ~86us v1): the main loop is bound by the PSUM->engine
read port (32b/cycle/lane on DVE and ACT; GPSIMD and DMA cannot touch PSUM),
so everything else is folded away:
- The Schraudolph exp transform (i16 = round(sim*1024*log2e + 15*1024 - C),
  bitcast to fp16) is fused INTO the qk matmul: q is pre-scaled by
  A10*SCALE on the host and a 12th constant feature (k=1, q=b10c) adds the
  offset. Both DVE and ACT then drain sim tiles with plain f32->i16
  round-convert copies (verified round-to-nearest on HW), costing exactly
  the PSUM port bound. ACT takes 8/15 tiles, DVE 7/15 (rate-balanced).
- v row-major obtained by tiled DMA xbar transposes ([32,1024] fp16 ->
  [128,8,32]) ALL on the sync queue (v1 put 2 on the scalar queue, which
  now must spend every cycle draining PSUM); explicit sync deps on both
  sides as before.
- keys packed 4 tiles/column-group at partition bases {0,32,64,96}; qk h0
  reads kvT on band b, qk h1 reads a 64-partition-rotated copy (kvT2) on
  band (b+2)%4, so both halves run concurrently on distinct (row-band,
  PSUM-bank) pairs. Concurrent row-tiled MMs into the SAME PSUM bank fault.
- av halves spread over all 4 col groups via (t + 2h) % 4; merged in the
  epilogue after a transpose, divided by the fused denominator column.
- single shared PSUM pool (3x [128,1024] f32 sim slots + out_big) from t=0
  so prologue transposes interleave with the first qk tiles; consts are
  uploaded pre-cast fp16; x arrives in 4 chunks on 3 DMA queues.
"""

import ml_dtypes
import numpy as np

import concourse.bass as bass
import concourse.bacc as bacc
from concourse import mybir
from concourse.tile import TileContext
from concourse.tile_rust import add_dep_helper
from concourse.bass_utils import run_bass_kernel_spmd

F32 = mybir.dt.float32
BF16 = mybir.dt.bfloat16
I16 = mybir.dt.int16

N = 8192
NCORES = 8
NQ = N // NCORES
P = 128
R = N // P
RQ = NQ // P
D = 10
DA = D + 1
KO = 11
KF = 12           # k features incl. the b10c-partner constant
VA = 12           # v features incl. the denominator-ones column
V_OFF = 12        # v offset inside each 32-wide kv band
VSS = 32          # vS slot stride (full transposed kv block)
NM = R // 4
NC = NM * P
NJ = 64
EPS = 1e-5
SCALE = D ** -0.5

LOG2E = 1.4426950408889634
A10 = 128.0 * LOG2E
C_TUNED = 7.625

# fp16 consts: identH | wkvB | wq (12)
CWH = P + P + KF


def _build_nc():
    nc = bacc.Bacc(None, target_bir_lowering=False)

    x_d = nc.dram_tensor("x", [N, D], F32, kind="ExternalInput")
    xq_d = nc.dram_tensor("xq", [NQ, D], F32, kind="ExternalInput")
    csth_d = nc.dram_tensor("csth", [P, CWH], BF16, kind="ExternalInput")
    cst32_d = nc.dram_tensor("cst32", [P, VA], BF16, kind="ExternalInput")
    y_d = nc.dram_tensor("y", [NQ, KO], F32, kind="ExternalOutput")

    with TileContext(nc) as tc:
        with (
            tc.tile_pool(name="const", bufs=1) as constp,
            tc.tile_pool(name="big", bufs=1) as bigp,
        ):
            csth = constp.tile([P, CWH], BF16)
            nc.sync.dma_start(out=csth, in_=csth_d[:])
            identH = csth[:, 0:P]
            wkvh = csth[:, P : 2 * P]
            wqh = csth[0:DA, 2 * P : 2 * P + KF]

            # 4-group merge matrix for the epilogue matmul-merge
            mrg = constp.tile([P, VA], BF16)

            xnT = bigp.tile([P, NC], BF16)
            kvT = bigp.tile([P, NC], BF16)
            kvT2 = bigp.tile([P, NC], BF16)   # kvT rotated 64 partitions
            qR = bigp.tile([P, NQ], BF16)
            vS = bigp.tile([P, NJ * VSS], BF16)  # transposed kv blocks
            xqT = bigp.tile([DA, NQ], BF16)

            with tc.tile_pool(name="work", bufs=1) as workp:
                pstp_cm = tc.tile_pool(name="pst", bufs=2, space="PSUM")
                pstp = pstp_cm.__enter__()

                def mk_pt():
                    return pstp.tile([P, 512], BF16, name="ptx", tag="ps")

                def mk_pk():
                    return pstp.tile([P, 512], F32, name="pk", tag="ps")

                def ln_stats(xr, nrows_p, name, sq_on_act=False):
                    """stats chain -> (mu, tenvar=10*var)."""
                    sq = workp.tile([P, nrows_p, D], F32, name=f"sq_{name}")
                    if sq_on_act:
                        nc.scalar.activation(
                            out=sq, in_=xr,
                            func=mybir.ActivationFunctionType.Square,
                            bias=0.0, scale=1.0)
                    else:
                        nc.vector.tensor_mul(sq, xr, xr)
                    s1 = workp.tile([P, nrows_p], F32, name=f"s1_{name}")
                    nc.vector.reduce_sum(out=s1, in_=xr, axis=mybir.AxisListType.X)
                    tv = workp.tile([P, nrows_p], F32, name=f"tv_{name}")
                    nc.vector.reduce_sum(out=tv, in_=sq, axis=mybir.AxisListType.X)
                    mu = workp.tile([P, nrows_p], F32, name=f"mu_{name}")
                    nc.vector.tensor_scalar_mul(mu, s1, 1.0 / D)
                    musq = workp.tile([P, nrows_p], F32, name=f"musq_{name}")
                    nc.vector.tensor_mul(musq, mu, s1)
                    nc.vector.tensor_sub(tv, tv, musq)
                    return mu, tv

                def rsig_of(tv, nrows_p, name):
                    """1/sqrt(tenvar/10 + eps): ACT Sqrt + DVE reciprocal."""
                    sg = workp.tile([P, nrows_p], F32, name=f"sg_{name}")
                    nc.scalar.activation(
                        out=sg, in_=tv,
                        func=mybir.ActivationFunctionType.Sqrt,
                        bias=eps, scale=1.0 / D)
                    rs = workp.tile([P, nrows_p], F32, name=f"rs_{name}")
                    nc.vector.reciprocal(rs, sg)
                    return rs

                def ln_finish(xr, xa_slice, mu, rsig, nrows_p, sub,
                              eng=None):
                    e = eng or nc.vector
                    for h0 in range(0, nrows_p, sub):
                        h1 = min(h0 + sub, nrows_p)
                        nh = h1 - h0
                        e.tensor_sub(
                            xa_slice[:, h0:h1, 0:D], xr[:, h0:h1, :],
                            mu[:, h0:h1].broadcast_to([P, nh, D]),
                        )
                        e.tensor_mul(
                            xa_slice[:, h0:h1, 0:D], xa_slice[:, h0:h1, 0:D],
                            rsig[:, h0:h1].broadcast_to([P, nh, D]),
                        )
                        e.memset(xa_slice[:, h0:h1, D : D + 1], 1.0)

                # ---------- DMAs (before any ACT op: the sqrt table load
                # otherwise blocks the scalar DMA queue for ~2.6us) ----------
                xq_r = workp.tile([P, RQ, D], F32, name="xr_q")
                xq_src = xq_d.rearrange("(p r) c -> p r c", p=P)
                nc.scalar.dma_start(out=xq_r[:, 0:4, :], in_=xq_src[:, 0:4, :])
                nc.sync.dma_start(out=xq_r[:, 4:8, :], in_=xq_src[:, 4:8, :])
                x_r = workp.tile([P, R, D], F32, name="xr_x")
                x_src = x_d.rearrange("(p r) c -> p r c", p=P)
                nc.sync.dma_start(out=x_r[:, 0:16, :], in_=x_src[:, 0:16, :])
                nc.gpsimd.dma_start(out=x_r[:, 16:32, :], in_=x_src[:, 16:32, :])
                nc.scalar.dma_start(out=x_r[:, 32:48, :], in_=x_src[:, 32:48, :])
                nc.scalar.dma_start(out=x_r[:, 48:64, :], in_=x_src[:, 48:64, :])
                # merge matrix only needed by the epilogue
                nc.scalar.dma_start(out=mrg, in_=cst32_d[:])

                eps = constp.tile([P, 1], F32)
                nc.vector.memset(eps, EPS)
                # dummy Sqrt pulls the sqrt table set load under the DMAs
                scr = constp.tile([P, 2], F32)
                nc.scalar.activation(
                    out=scr[:, 0:1], in_=eps,
                    func=mybir.ActivationFunctionType.Sqrt, bias=0.0, scale=1.0)
                # pad columns of the kv activations, all slices in one shot
                xa = workp.tile([P, R, 32], BF16, name="xa_x")
                nc.gpsimd.memset(xa[:, :, DA:32], 0.0)

                # ---------- x slice 0 first: its data arrives first and
                # it gates the first qk quad's kvT chunk ----------
                x_stats = {}

                def x_stats_for(s):
                    xrs = x_r[:, 16 * s : 16 * s + 16, :]
                    mu, tv = ln_stats(xrs, 16, f"x{s}", sq_on_act=True)
                    rs = rsig_of(tv, 16, f"x{s}")
                    x_stats[s] = (mu, rs)

                def x_stats_for23():
                    mu, tv = ln_stats(x_r[:, 32:64, :], 32, "x23",
                                      sq_on_act=True)
                    rs = rsig_of(tv, 32, "x23")
                    x_stats[2] = (mu[:, 0:16], rs[:, 0:16])
                    x_stats[3] = (mu[:, 16:32], rs[:, 16:32])

                qrdma = [None] * 4

                def q_side():
                    q_mu, q_tv = ln_stats(xq_r, RQ, "q")
                    q_rs = rsig_of(q_tv, RQ, "q")
                    xqa = workp.tile([P, RQ, DA], BF16, name="xa_q")
                    ln_finish(xq_r, xqa, q_mu, q_rs, RQ, RQ, eng=nc.gpsimd)
                    for g in range(RQ // 4):
                        pt = mk_pt()
                        for k4 in range(4):
                            r = g * 4 + k4
                            nc.tensor.transpose(
                                pt[0:DA, k4 * P : (k4 + 1) * P], xqa[:, r, :],
                                identH,
                            )
                        nc.vector.tensor_copy(
                            xqT[:, g * 512 : (g + 1) * 512], pt[0:DA, :]
                        )
                    pqcopies = []
                    for t in range(NQ // 512):
                        pq = mk_pk()
                        nc.tensor.matmul(
                            pq[0:KF, :], wqh, xqT[:, t * 512 : (t + 1) * 512],
                            start=True, stop=True,
                        )
                        if t % 2 == 0:
                            pqcopies.append(nc.vector.tensor_copy(
                                qR[0:KF, t * 512 : (t + 1) * 512], pq[0:KF, :]))
                        else:
                            pqcopies.append(nc.scalar.copy(
                                qR[0:KF, t * 512 : (t + 1) * 512], pq[0:KF, :]))
                    for bi, rp in enumerate((32, 64, 96)):
                        qrdma[bi + 1] = nc.sync.dma_start(
                            out=qR[rp : rp + KF, :], in_=qR[0:KF, :])
                        for cp_ in pqcopies:
                            add_dep_helper(qrdma[bi + 1].ins, cp_.ins, sync=True,
                                           reason="qR replicate after pq copies")

                # explicit sync deps: the DMA-written kvT2/vS consumers race
                # without them (dep tracking misses the strided DMA outputs)
                k2dma = [[None] * 4 for _ in range(4)]
                vdma = [[None] * 4 for _ in range(2)]
                kvcopy = [None] * 4

                # ---------- per-slice normalize/transpose/proj/kvT/v ------
                def x_slice(s):
                    r0 = 16 * s
                    mu, rs = x_stats[s]
                    xah = xa[:, r0 : r0 + 16, :]
                    # NOTE: ln_finish must NOT share the gpsimd queue with
                    # the k2dma descriptor gens -- the scheduler interleaves
                    # them into a cross-engine FIFO cycle (14us stall).
                    ln_finish(x_r[:, r0 : r0 + 16, :], xah, mu, rs, 16, 16,
                              eng=nc.vector)
                    ch = s
                    pt = mk_pt()
                    for mi in range(4):
                        m = ch * 4 + mi
                        nc.tensor.transpose(
                            pt[:, mi * P : (mi + 1) * P],
                            xa[:, m * 4 : m * 4 + 4, :], identH,
                        )
                    dst = xnT[:, ch * 512 : (ch + 1) * 512]
                    if ch % 2 == 0:
                        nc.vector.tensor_copy(dst, pt)
                    else:
                        nc.scalar.copy(dst, pt)
                    pk = mk_pk()
                    nc.tensor.matmul(
                        pk, wkvh, xnT[:, ch * 512 : (ch + 1) * 512],
                        start=True, stop=True,
                    )
                    dstk = kvT[:, ch * 512 : (ch + 1) * 512]
                    if ch % 2 == 0:
                        kvcopy[ch] = nc.scalar.copy(dstk, pk)
                    else:
                        kvcopy[ch] = nc.vector.tensor_copy(dstk, pk)
                    # rotate the k rows by 64 partitions so qk h1 can use
                    # row band (b+2)%4: both qk MMs of a tile run
                    # concurrently. gpsimd DMA queue is idle here.
                    cs = slice(ch * 512, (ch + 1) * 512)
                    for b in range(4):
                        bb = (b + 2) % 4
                        k2dma[ch][bb] = nc.gpsimd.dma_start(
                            out=kvT2[32 * bb : 32 * bb + KF, cs],
                            in_=kvT[32 * b : 32 * b + KF, cs])
                        add_dep_helper(k2dma[ch][bb].ins, kvcopy[ch].ins,
                                       sync=True, reason="kvT2 dma after copy")

                def v_half(h):
                    # v row-major: one tiled DMA xbar transpose per base --
                    # in [32, 1024] -> out [128, 8, 32] (8 kv blocks)
                    vS_r4 = vS.rearrange("p (m b c) -> p m b c", b=4, c=VSS)
                    for b in range(4):
                        vdma[h][b] = nc.sync.dma_start_transpose(
                            out=vS_r4[:, h * 8 : (h + 1) * 8, b, :],
                            in_=kvT[32 * b : 32 * b + 32,
                                    h * 1024 : (h + 1) * 1024],
                        )
                        add_dep_helper(vdma[h][b].ins, kvcopy[2 * h].ins,
                                       sync=True, reason="v dma after kv copies")
                        add_dep_helper(vdma[h][b].ins, kvcopy[2 * h + 1].ins,
                                       sync=True, reason="v dma after kv copies")

                q_side()
                x_stats_for(0)
                x_slice(0)
                x_stats_for(1)
                x_slice(1)
                v_half(0)
                x_stats_for23()
                x_slice(2)
                x_slice(3)
                v_half(1)
                pstp_cm.__exit__(None, None, None)

                # ---------- attention main loop ----------
                outp_cm = tc.tile_pool(name="outp", bufs=1, space="PSUM")
                outp = outp_cm.__enter__()
                out_big = outp.tile([P, NQ], F32)
                simp_cm = tc.tile_pool(name="simp", bufs=3, space="PSUM")
                simp = simp_cm.__enter__()

                def mk_sim():
                    return simp.tile([P, NQ], F32, name="sim")

                with tc.tile_pool(name="expp", bufs=12) as expp:

                    def emit_av(t, et):
                        vj = vS[:, t * VSS + V_OFF : t * VSS + V_OFF + VA]
                        vd = vdma[t // 32][t % 4]
                        for hh in range(NQ // 512):
                            cp = ((t + 2 * hh) % 4) * 32
                            mm = nc.tensor.matmul(
                                out_big[cp : cp + VA, hh * 512 : (hh + 1) * 512],
                                vj, et[:, hh * 512 : (hh + 1) * 512],
                                start=(t < 2), stop=(t >= NJ - 2),
                                tile_position=(0, cp),
                            )
                            if ("v", t // 32, t % 4) not in dep_done:
                                dep_done.add(("v", t // 32, t % 4))
                                add_dep_helper(mm.ins, vd.ins, sync=True,
                                               reason="av after v xbar dma")

                    dep_done = set()

                    def emit_qk(t):
                        m, b = t // 4, t % 4
                        sim = mk_sim()
                        for hh in range(NQ // 512):
                            # h1 reads the rotated copy on band (b+2)%4
                            bb = (b + 2 * hh) % 4
                            rp = bb * 32
                            src = kvT if hh == 0 else kvT2
                            qk = nc.tensor.matmul(
                                sim[:, hh * 512 : (hh + 1) * 512],
                                src[rp : rp + KF, m * P : (m + 1) * P],
                                qR[rp : rp + KF, hh * 512 : (hh + 1) * 512],
                                start=True, stop=True,
                                tile_position=(rp, 0),
                            )
                            if bb > 0 and ("qr", bb) not in dep_done:
                                dep_done.add(("qr", bb))
                                add_dep_helper(
                                    qk.ins, qrdma[bb].ins,
                                    sync=True, reason="qk after qR replicate")
                            if hh == 1 and ("k2", t // 16, bb) not in dep_done:
                                dep_done.add(("k2", t // 16, bb))
                                add_dep_helper(
                                    qk.ins, k2dma[t // 16][bb].ins,
                                    sync=True, reason="qk h1 after kvT2 dma")
                        return sim

                    def emit_drain(t, sim):
                        et = expp.tile([P, NQ], BF16, name="et")
                        # fused Schraudolph: sim already = A10*qk + b10c;
                        # f32->i16 convert-copy rounds to nearest (HW-checked).
                        # ACT is faster per tile: 5:3 ACT-heavy pattern
                        if t % 8 in (0, 3, 5):
                            nc.vector.tensor_copy(et[:].bitcast(I16), sim)
                        else:
                            nc.scalar.copy(et[:].bitcast(I16), sim)
                        return et

                    # batches of 2 tiles: the 4 qk MMs cover all 4 row
                    # bands and sit adjacent in the PE FIFO (concurrent);
                    # same for the 4 av MMs over all 4 col groups
                    pend = []
                    for g in range(NJ // 2):
                        ts = (2 * g, 2 * g + 1)
                        sims = [emit_qk(t) for t in ts]
                        ets = [emit_drain(t, s) for t, s in zip(ts, sims)]
                        pend.append((ts, ets))
                        if len(pend) > 2:
                            ts_, ets_ = pend.pop(0)
                            for t_, e_ in zip(ts_, ets_):
                                emit_av(t_, e_)
                    for ts_, ets_ in pend:
                        for t_, e_ in zip(ts_, ets_):
                            emit_av(t_, e_)

                # ---------- epilogue ----------
                simp_cm.__exit__(None, None, None)
                with tc.tile_pool(name="ep", bufs=1) as epp, \
                     tc.tile_pool(name="epps", bufs=1, space="PSUM") as eppsp:
                    MW = 108
                    oS = epp.tile([P, NQ], BF16)
                    nc.vector.tensor_copy(oS[0:MW, 0:512], out_big[0:MW, 0:512])
                    nc.scalar.copy(oS[0:MW, 512:1024], out_big[0:MW, 512:1024])
                    # transpose + 4-group merge in one matmul per q-chunk:
                    # out[q, f] = sum_p oS[p, q] * mrg[p, f]
                    pm = eppsp.tile([P, RQ * VA], F32, name="pm")
                    for ch in range(RQ):
                        nc.tensor.matmul(
                            pm[:, ch * VA : (ch + 1) * VA],
                            oS[0:MW, ch * P : (ch + 1) * P],
                            mrg[0:MW, :],
                            start=True, stop=True,
                        )
                    oM = epp.tile([P, RQ, VA], F32)
                    nc.vector.tensor_copy(
                        oM, pm.rearrange("p (t c) -> p t c", c=VA))
                    rec = epp.tile([P, RQ], F32)
                    nc.vector.reciprocal(rec, oM[:, :, KO])
                    oF = epp.tile([P, RQ, KO], F32)
                    nc.vector.tensor_mul(
                        oF, oM[:, :, 0:KO], rec.broadcast_to([P, RQ, KO])
                    )
                    nc.scalar.dma_start(
                        out=y_d.rearrange("(p t) c -> p t c", p=P), in_=oF
                    )
                outp_cm.__exit__(None, None, None)
    nc.compile()
    return nc


_NC_CACHE = {}


def _get_nc():
    if "nc" not in _NC_CACHE:
        _NC_CACHE["nc"] = _build_nc()
    return _NC_CACHE["nc"]


def _host_prep(x, gamma, beta, W):
    x = np.asarray(x, np.float32)
    gamma = np.asarray(gamma, np.float32)
    beta = np.asarray(beta, np.float32)
    W = np.asarray(W, np.float32)
    Wg = W * gamma[None, :]
    b0 = W @ beta
    Wq, Wk, Wv = Wg[0:KO], Wg[KO : 2 * KO], Wg[2 * KO : 3 * KO]
    bq, bk, bv = b0[0:KO], b0[KO : 2 * KO], b0[2 * KO : 3 * KO]

    b10c = 127.0 * 128.0 - C_TUNED
    a10s = A10 * SCALE

    # block-diagonal kv projection, per 32-band: 11 k features, then the
    # constant-1 partner of the b10c offset, then 11 v features + the
    # denominator-ones column at V_OFF..V_OFF+VA
    wkvB = np.zeros((P, P), np.float32)
    for b in range(4):
        o = 32 * b
        wkvB[o : o + D, o : o + KO] = Wk.T
        wkvB[o + D, o : o + KO] = bk
        wkvB[o + D, o + KO] = 1.0
        wkvB[o : o + D, o + V_OFF : o + V_OFF + KO] = Wv.T
        wkvB[o + D, o + V_OFF : o + V_OFF + KO] = bv
        wkvB[o + D, o + V_OFF + KO] = 1.0

    # q projection pre-scaled by A10*SCALE; col 11 emits the b10c constant
    wq_a = np.zeros((DA, KF), np.float32)
    wq_a[0:D, 0:KO] = Wq.T * a10s
    wq_a[D, 0:KO] = bq * a10s
    wq_a[D, KO] = b10c

    csth = np.zeros((P, CWH), np.float32)
    csth[:, 0:P] = np.eye(P)
    csth[:, P : 2 * P] = wkvB
    csth[0:DA, 2 * P : 2 * P + KF] = wq_a
    csth = csth.astype(ml_dtypes.bfloat16)
    # merge matrix: sum the 4 column-group partials during the epilogue
    # transpose-matmul; out[q, f] = sum_g out_big[32g + f, q]
    cst32 = np.zeros((P, VA), np.float32)
    for g in range(4):
        for f in range(VA):
            cst32[32 * g + f, f] = 1.0
    cst32 = cst32.astype(ml_dtypes.bfloat16)
    return x, csth, cst32


def _run(x, gamma, beta, W, **spmd_kwargs):
    nc = _get_nc()
    x, csth, cst32 = _host_prep(x, gamma, beta, W)
    in_maps = []
    for c in range(NCORES):
        in_maps.append({
            "x": x,
            "xq": np.ascontiguousarray(x[c * NQ : (c + 1) * NQ]),
            "csth": csth,
            "cst32": cst32,
        })
    res = run_bass_kernel_spmd(
        nc, in_maps, core_ids=list(range(NCORES)), **spmd_kwargs
    )
    out = np.concatenate([res.results[c]["y"] for c in range(NCORES)], axis=0)
    return out, res


def kernel(x, gamma, beta, W):
    out, _ = _run(x, gamma, beta, W)
    return out


# revision 39
# speedup vs baseline: 1.0120x; 1.0066x over previous
"""Sequence-parallel fused LayerNorm + QKV-projection + attention for TRN2.

x [8192,10] f32 -> LN -> h @ W.T -> q,k,v -> out = softmax(q k^T) v [8192,11].
The 8192 query rows are sharded across 8 NeuronCores (1024 each); every core
computes k/v for all keys itself (projection is tiny, no collectives).

v2 design (from the ~86us v1): the main loop is bound by the PSUM->engine
read port (32b/cycle/lane on DVE and ACT; GPSIMD and DMA cannot touch PSUM),
so everything else is folded away:
- The Schraudolph exp transform (i16 = round(sim*1024*log2e + 15*1024 - C),
  bitcast to fp16) is fused INTO the qk matmul: q is pre-scaled by
  A10*SCALE on the host and a 12th constant feature (k=1, q=b10c) adds the
  offset. Both DVE and ACT then drain sim tiles with plain f32->i16
  round-convert copies (verified round-to-nearest on HW), costing exactly
  the PSUM port bound. ACT takes 8/15 tiles, DVE 7/15 (rate-balanced).
- v row-major obtained by tiled DMA xbar transposes ([32,1024] fp16 ->
  [128,8,32]) ALL on the sync queue (v1 put 2 on the scalar queue, which
  now must spend every cycle draining PSUM); explicit sync deps on both
  sides as before.
- keys packed 4 tiles/column-group at partition bases {0,32,64,96}; qk h0
  reads kvT on band b, qk h1 reads a 64-partition-rotated copy (kvT2) on
  band (b+2)%4, so both halves run concurrently on distinct (row-band,
  PSUM-bank) pairs. Concurrent row-tiled MMs into the SAME PSUM bank fault.
- av halves spread over all 4 col groups via (t + 2h) % 4; merged in the
  epilogue after a transpose, divided by the fused denominator column.
- single shared PSUM pool (3x [128,1024] f32 sim slots + out_big) from t=0
  so prologue transposes interleave with the first qk tiles; consts are
  uploaded pre-cast fp16; x arrives in 4 chunks on 3 DMA queues.
"""

import ml_dtypes
import numpy as np

import concourse.bass as bass
import concourse.bacc as bacc
from concourse import mybir
from concourse.tile import TileContext
from concourse.tile_rust import add_dep_helper
from concourse.bass_utils import run_bass_kernel_spmd

F32 = mybir.dt.float32
BF16 = mybir.dt.bfloat16
I16 = mybir.dt.int16

N = 8192
NCORES = 8
NQ = N // NCORES
P = 128
R = N // P
RQ = NQ // P
D = 10
DA = D + 1
KO = 11
KF = 12           # k features incl. the b10c-partner constant
VA = 12           # v features incl. the denominator-ones column
V_OFF = 12        # v offset inside each 32-wide kv band
VSS = 32          # vS slot stride (full transposed kv block)
NM = R // 4
NC = NM * P
NJ = 64
EPS = 1e-5
SCALE = D ** -0.5

LOG2E = 1.4426950408889634
A10 = 128.0 * LOG2E
C_TUNED = 7.625

# fp16 consts: identH | wkvB | wq (12)
CWH = P + P + KF


def _build_nc():
    nc = bacc.Bacc(None, target_bir_lowering=False)

    x_d = nc.dram_tensor("x", [N, D], F32, kind="ExternalInput")
    xq_d = nc.dram_tensor("xq", [NQ, D], F32, kind="ExternalInput")
    csth_d = nc.dram_tensor("csth", [P, CWH], BF16, kind="ExternalInput")
    cst32_d = nc.dram_tensor("cst32", [P, VA], BF16, kind="ExternalInput")
    y_d = nc.dram_tensor("y", [NQ, KO], F32, kind="ExternalOutput")

    with TileContext(nc) as tc:
        with (
            tc.tile_pool(name="const", bufs=1) as constp,
            tc.tile_pool(name="big", bufs=1) as bigp,
        ):
            csth = constp.tile([P, CWH], BF16)
            nc.sync.dma_start(out=csth, in_=csth_d[:])
            identH = csth[:, 0:P]
            wkvh = csth[:, P : 2 * P]
            wqh = csth[0:DA, 2 * P : 2 * P + KF]

            # 4-group merge matrix for the epilogue matmul-merge
            mrg = constp.tile([P, VA], BF16)

            xnT = bigp.tile([P, NC], BF16)
            kvT = bigp.tile([P, NC], BF16)
            kvT2 = bigp.tile([P, NC], BF16)   # kvT rotated 64 partitions
            qR = bigp.tile([P, NQ], BF16)
            vS = bigp.tile([P, NJ * VSS], BF16)  # transposed kv blocks
            xqT = bigp.tile([DA, NQ], BF16)

            with tc.tile_pool(name="work", bufs=1) as workp:
                pstp_cm = tc.tile_pool(name="pst", bufs=2, space="PSUM")
                pstp = pstp_cm.__enter__()

                def mk_pt():
                    return pstp.tile([P, 512], BF16, name="ptx", tag="ps")

                def mk_pk():
                    return pstp.tile([P, 512], F32, name="pk", tag="ps")

                def ln_stats(xr, nrows_p, name, sq_on_act=False):
                    """stats chain -> (mu, tenvar=10*var)."""
                    sq = workp.tile([P, nrows_p, D], F32, name=f"sq_{name}")
                    if sq_on_act:
                        nc.scalar.activation(
                            out=sq, in_=xr,
                            func=mybir.ActivationFunctionType.Square,
                            bias=0.0, scale=1.0)
                    else:
                        nc.vector.tensor_mul(sq, xr, xr)
                    s1 = workp.tile([P, nrows_p], F32, name=f"s1_{name}")
                    nc.vector.reduce_sum(out=s1, in_=xr, axis=mybir.AxisListType.X)
                    tv = workp.tile([P, nrows_p], F32, name=f"tv_{name}")
                    nc.vector.reduce_sum(out=tv, in_=sq, axis=mybir.AxisListType.X)
                    mu = workp.tile([P, nrows_p], F32, name=f"mu_{name}")
                    nc.vector.tensor_scalar_mul(mu, s1, 1.0 / D)
                    musq = workp.tile([P, nrows_p], F32, name=f"musq_{name}")
                    nc.vector.tensor_mul(musq, mu, s1)
                    nc.vector.tensor_sub(tv, tv, musq)
                    return mu, tv

                def rsig_of(tv, nrows_p, name):
                    """1/sqrt(tenvar/10 + eps): ACT Sqrt + DVE reciprocal."""
                    sg = workp.tile([P, nrows_p], F32, name=f"sg_{name}")
                    nc.scalar.activation(
                        out=sg, in_=tv,
                        func=mybir.ActivationFunctionType.Sqrt,
                        bias=eps, scale=1.0 / D)
                    rs = workp.tile([P, nrows_p], F32, name=f"rs_{name}")
                    nc.vector.reciprocal(rs, sg)
                    return rs

                def ln_finish(xr, xa_slice, mu, rsig, nrows_p, sub,
                              eng=None):
                    e = eng or nc.vector
                    for h0 in range(0, nrows_p, sub):
                        h1 = min(h0 + sub, nrows_p)
                        nh = h1 - h0
                        e.tensor_sub(
                            xa_slice[:, h0:h1, 0:D], xr[:, h0:h1, :],
                            mu[:, h0:h1].broadcast_to([P, nh, D]),
                        )
                        e.tensor_mul(
                            xa_slice[:, h0:h1, 0:D], xa_slice[:, h0:h1, 0:D],
                            rsig[:, h0:h1].broadcast_to([P, nh, D]),
                        )
                        e.memset(xa_slice[:, h0:h1, D : D + 1], 1.0)

                # ---------- DMAs (before any ACT op: the sqrt table load
                # otherwise blocks the scalar DMA queue for ~2.6us) ----------
                xq_r = workp.tile([P, RQ, D], F32, name="xr_q")
                xq_src = xq_d.rearrange("(p r) c -> p r c", p=P)
                nc.scalar.dma_start(out=xq_r[:, 0:4, :], in_=xq_src[:, 0:4, :])
                nc.sync.dma_start(out=xq_r[:, 4:8, :], in_=xq_src[:, 4:8, :])
                x_r = workp.tile([P, R, D], F32, name="xr_x")
                x_src = x_d.rearrange("(p r) c -> p r c", p=P)
                nc.sync.dma_start(out=x_r[:, 0:16, :], in_=x_src[:, 0:16, :])
                nc.gpsimd.dma_start(out=x_r[:, 16:32, :], in_=x_src[:, 16:32, :])
                nc.scalar.dma_start(out=x_r[:, 32:48, :], in_=x_src[:, 32:48, :])
                nc.scalar.dma_start(out=x_r[:, 48:64, :], in_=x_src[:, 48:64, :])
                # merge matrix only needed by the epilogue
                nc.scalar.dma_start(out=mrg, in_=cst32_d[:])

                eps = constp.tile([P, 1], F32)
                nc.vector.memset(eps, EPS)
                # dummy Sqrt pulls the sqrt table set load under the DMAs
                scr = constp.tile([P, 2], F32)
                nc.scalar.activation(
                    out=scr[:, 0:1], in_=eps,
                    func=mybir.ActivationFunctionType.Sqrt, bias=0.0, scale=1.0)
                # pad columns of the kv activations, all slices in one shot
                xa = workp.tile([P, R, 32], BF16, name="xa_x")
                nc.gpsimd.memset(xa[:, :, DA:32], 0.0)

                # ---------- x slice 0 first: its data arrives first and
                # it gates the first qk quad's kvT chunk ----------
                x_stats = {}

                def x_stats_for(s):
                    xrs = x_r[:, 16 * s : 16 * s + 16, :]
                    mu, tv = ln_stats(xrs, 16, f"x{s}", sq_on_act=True)
                    rs = rsig_of(tv, 16, f"x{s}")
                    x_stats[s] = (mu, rs)

                def x_stats_for23():
                    mu, tv = ln_stats(x_r[:, 32:64, :], 32, "x23",
                                      sq_on_act=True)
                    rs = rsig_of(tv, 32, "x23")
                    x_stats[2] = (mu[:, 0:16], rs[:, 0:16])
                    x_stats[3] = (mu[:, 16:32], rs[:, 16:32])

                qrdma = [None] * 4

                def q_side():
                    q_mu, q_tv = ln_stats(xq_r, RQ, "q")
                    q_rs = rsig_of(q_tv, RQ, "q")
                    xqa = workp.tile([P, RQ, DA], BF16, name="xa_q")
                    ln_finish(xq_r, xqa, q_mu, q_rs, RQ, RQ, eng=nc.gpsimd)
                    for g in range(RQ // 4):
                        pt = mk_pt()
                        for k4 in range(4):
                            r = g * 4 + k4
                            nc.tensor.transpose(
                                pt[0:DA, k4 * P : (k4 + 1) * P], xqa[:, r, :],
                                identH,
                            )
                        nc.vector.tensor_copy(
                            xqT[:, g * 512 : (g + 1) * 512], pt[0:DA, :]
                        )
                    pqcopies = []
                    for t in range(NQ // 512):
                        pq = mk_pk()
                        nc.tensor.matmul(
                            pq[0:KF, :], wqh, xqT[:, t * 512 : (t + 1) * 512],
                            start=True, stop=True,
                        )
                        if t % 2 == 0:
                            pqcopies.append(nc.vector.tensor_copy(
                                qR[0:KF, t * 512 : (t + 1) * 512], pq[0:KF, :]))
                        else:
                            pqcopies.append(nc.scalar.copy(
                                qR[0:KF, t * 512 : (t + 1) * 512], pq[0:KF, :]))
                    for bi, rp in enumerate((32, 64, 96)):
                        qrdma[bi + 1] = nc.sync.dma_start(
                            out=qR[rp : rp + KF, :], in_=qR[0:KF, :])
                        for cp_ in pqcopies:
                            add_dep_helper(qrdma[bi + 1].ins, cp_.ins, sync=True,
                                           reason="qR replicate after pq copies")

                # explicit sync deps: the DMA-written kvT2/vS consumers race
                # without them (dep tracking misses the strided DMA outputs)
                k2dma = [[None] * 4 for _ in range(4)]
                vdma = [[None] * 4 for _ in range(2)]
                kvcopy = [None] * 4

                # ---------- per-slice normalize/transpose/proj/kvT/v ------
                def x_slice(s):
                    r0 = 16 * s
                    mu, rs = x_stats[s]
                    xah = xa[:, r0 : r0 + 16, :]
                    # NOTE: ln_finish must NOT share the gpsimd queue with
                    # the k2dma descriptor gens -- the scheduler interleaves
                    # them into a cross-engine FIFO cycle (14us stall).
                    ln_finish(x_r[:, r0 : r0 + 16, :], xah, mu, rs, 16, 16,
                              eng=nc.vector)
                    ch = s
                    pt = mk_pt()
                    for mi in range(4):
                        m = ch * 4 + mi
                        nc.tensor.transpose(
                            pt[:, mi * P : (mi + 1) * P],
                            xa[:, m * 4 : m * 4 + 4, :], identH,
                        )
                    dst = xnT[:, ch * 512 : (ch + 1) * 512]
                    if ch % 2 == 0:
                        nc.vector.tensor_copy(dst, pt)
                    else:
                        nc.scalar.copy(dst, pt)
                    pk = mk_pk()
                    nc.tensor.matmul(
                        pk, wkvh, xnT[:, ch * 512 : (ch + 1) * 512],
                        start=True, stop=True,
                    )
                    dstk = kvT[:, ch * 512 : (ch + 1) * 512]
                    if ch % 2 == 0:
                        kvcopy[ch] = nc.scalar.copy(dstk, pk)
                    else:
                        kvcopy[ch] = nc.vector.tensor_copy(dstk, pk)
                    # rotate the k rows by 64 partitions so qk h1 can use
                    # row band (b+2)%4: both qk MMs of a tile run
                    # concurrently. gpsimd DMA queue is idle here.
                    cs = slice(ch * 512, (ch + 1) * 512)
                    for b in range(4):
                        bb = (b + 2) % 4
                        k2dma[ch][bb] = nc.gpsimd.dma_start(
                            out=kvT2[32 * bb : 32 * bb + KF, cs],
                            in_=kvT[32 * b : 32 * b + KF, cs])
                        add_dep_helper(k2dma[ch][bb].ins, kvcopy[ch].ins,
                                       sync=True, reason="kvT2 dma after copy")

                def v_half(h):
                    # v row-major: one tiled DMA xbar transpose per base --
                    # in [32, 1024] -> out [128, 8, 32] (8 kv blocks)
                    vS_r4 = vS.rearrange("p (m b c) -> p m b c", b=4, c=VSS)
                    for b in range(4):
                        vdma[h][b] = nc.sync.dma_start_transpose(
                            out=vS_r4[:, h * 8 : (h + 1) * 8, b, :],
                            in_=kvT[32 * b : 32 * b + 32,
                                    h * 1024 : (h + 1) * 1024],
                        )
                        add_dep_helper(vdma[h][b].ins, kvcopy[2 * h].ins,
                                       sync=True, reason="v dma after kv copies")
                        add_dep_helper(vdma[h][b].ins, kvcopy[2 * h + 1].ins,
                                       sync=True, reason="v dma after kv copies")

                q_side()
                x_stats_for(0)
                x_slice(0)
                x_stats_for(1)
                x_slice(1)
                v_half(0)
                x_stats_for23()
                x_slice(2)
                x_slice(3)
                v_half(1)
                pstp_cm.__exit__(None, None, None)

                # ---------- attention main loop ----------
                outp_cm = tc.tile_pool(name="outp", bufs=1, space="PSUM")
                outp = outp_cm.__enter__()
                out_big = outp.tile([P, NQ], F32)
                simp_cm = tc.tile_pool(name="simp", bufs=3, space="PSUM")
                simp = simp_cm.__enter__()

                def mk_sim():
                    return simp.tile([P, NQ], F32, name="sim")

                with tc.tile_pool(name="expp", bufs=12) as expp:

                    def emit_av(t, et):
                        vj = vS[:, t * VSS + V_OFF : t * VSS + V_OFF + VA]
                        vd = vdma[t // 32][t % 4]
                        for hh in range(NQ // 512):
                            cp = ((t + 2 * hh) % 4) * 32
                            mm = nc.tensor.matmul(
                                out_big[cp : cp + VA, hh * 512 : (hh + 1) * 512],
                                vj, et[:, hh * 512 : (hh + 1) * 512],
                                start=(t < 2), stop=(t >= NJ - 2),
                                tile_position=(0, cp),
                            )
                            if ("v", t // 32, t % 4) not in dep_done:
                                dep_done.add(("v", t // 32, t % 4))
                                add_dep_helper(mm.ins, vd.ins, sync=True,
                                               reason="av after v xbar dma")

                    dep_done = set()

                    def emit_qk(t):
                        m, b = t // 4, t % 4
                        sim = mk_sim()
                        for hh in range(NQ // 512):
                            # h1 reads the rotated copy on band (b+2)%4
                            bb = (b + 2 * hh) % 4
                            rp = bb * 32
                            src = kvT if hh == 0 else kvT2
                            qk = nc.tensor.matmul(
                                sim[:, hh * 512 : (hh + 1) * 512],
                                src[rp : rp + KF, m * P : (m + 1) * P],
                                qR[rp : rp + KF, hh * 512 : (hh + 1) * 512],
                                start=True, stop=True,
                                tile_position=(rp, 0),
                            )
                            if bb > 0 and ("qr", bb) not in dep_done:
                                dep_done.add(("qr", bb))
                                add_dep_helper(
                                    qk.ins, qrdma[bb].ins,
                                    sync=True, reason="qk after qR replicate")
                            if hh == 1 and ("k2", t // 16, bb) not in dep_done:
                                dep_done.add(("k2", t // 16, bb))
                                add_dep_helper(
                                    qk.ins, k2dma[t // 16][bb].ins,
                                    sync=True, reason="qk h1 after kvT2 dma")
                        return sim

                    def emit_drain(t, sim):
                        et = expp.tile([P, NQ], BF16, name="et")
                        # fused Schraudolph: sim already = A10*qk + b10c;
                        # f32->i16 convert-copy rounds to nearest (HW-checked).
                        # ACT is faster per tile: 5:3 ACT-heavy pattern
                        if t % 8 in (0, 3, 5):
                            nc.vector.tensor_copy(et[:].bitcast(I16), sim)
                        else:
                            nc.scalar.copy(et[:].bitcast(I16), sim)
                        return et

                    # batches of 2 tiles: the 4 qk MMs cover all 4 row
                    # bands and sit adjacent in the PE FIFO (concurrent);
                    # same for the 4 av MMs over all 4 col groups
                    pend = []
                    for g in range(NJ // 2):
                        ts = (2 * g, 2 * g + 1)
                        sims = [emit_qk(t) for t in ts]
                        ets = [emit_drain(t, s) for t, s in zip(ts, sims)]
                        pend.append((ts, ets))
                        if len(pend) > 3:
                            ts_, ets_ = pend.pop(0)
                            for t_, e_ in zip(ts_, ets_):
                                emit_av(t_, e_)
                    for ts_, ets_ in pend:
                        for t_, e_ in zip(ts_, ets_):
                            emit_av(t_, e_)

                # ---------- epilogue ----------
                simp_cm.__exit__(None, None, None)
                with tc.tile_pool(name="ep", bufs=1) as epp, \
                     tc.tile_pool(name="epps", bufs=1, space="PSUM") as eppsp:
                    MW = 108
                    oS = epp.tile([P, NQ], BF16)
                    nc.vector.tensor_copy(oS[0:MW, 0:512], out_big[0:MW, 0:512])
                    nc.scalar.copy(oS[0:MW, 512:1024], out_big[0:MW, 512:1024])
                    # transpose + 4-group merge in one matmul per q-chunk:
                    # out[q, f] = sum_p oS[p, q] * mrg[p, f]
                    pm = eppsp.tile([P, RQ * VA], F32, name="pm")
                    for ch in range(RQ):
                        nc.tensor.matmul(
                            pm[:, ch * VA : (ch + 1) * VA],
                            oS[0:MW, ch * P : (ch + 1) * P],
                            mrg[0:MW, :],
                            start=True, stop=True,
                        )
                    oM = epp.tile([P, RQ, VA], F32)
                    nc.vector.tensor_copy(
                        oM, pm.rearrange("p (t c) -> p t c", c=VA))
                    rec = epp.tile([P, RQ], F32)
                    nc.vector.reciprocal(rec, oM[:, :, KO])
                    oF = epp.tile([P, RQ, KO], F32)
                    nc.vector.tensor_mul(
                        oF, oM[:, :, 0:KO], rec.broadcast_to([P, RQ, KO])
                    )
                    nc.scalar.dma_start(
                        out=y_d.rearrange("(p t) c -> p t c", p=P), in_=oF
                    )
                outp_cm.__exit__(None, None, None)
    nc.compile()
    return nc


_NC_CACHE = {}


def _get_nc():
    if "nc" not in _NC_CACHE:
        _NC_CACHE["nc"] = _build_nc()
    return _NC_CACHE["nc"]


def _host_prep(x, gamma, beta, W):
    x = np.asarray(x, np.float32)
    gamma = np.asarray(gamma, np.float32)
    beta = np.asarray(beta, np.float32)
    W = np.asarray(W, np.float32)
    Wg = W * gamma[None, :]
    b0 = W @ beta
    Wq, Wk, Wv = Wg[0:KO], Wg[KO : 2 * KO], Wg[2 * KO : 3 * KO]
    bq, bk, bv = b0[0:KO], b0[KO : 2 * KO], b0[2 * KO : 3 * KO]

    b10c = 127.0 * 128.0 - C_TUNED
    a10s = A10 * SCALE

    # block-diagonal kv projection, per 32-band: 11 k features, then the
    # constant-1 partner of the b10c offset, then 11 v features + the
    # denominator-ones column at V_OFF..V_OFF+VA
    wkvB = np.zeros((P, P), np.float32)
    for b in range(4):
        o = 32 * b
        wkvB[o : o + D, o : o + KO] = Wk.T
        wkvB[o + D, o : o + KO] = bk
        wkvB[o + D, o + KO] = 1.0
        wkvB[o : o + D, o + V_OFF : o + V_OFF + KO] = Wv.T
        wkvB[o + D, o + V_OFF : o + V_OFF + KO] = bv
        wkvB[o + D, o + V_OFF + KO] = 1.0

    # q projection pre-scaled by A10*SCALE; col 11 emits the b10c constant
    wq_a = np.zeros((DA, KF), np.float32)
    wq_a[0:D, 0:KO] = Wq.T * a10s
    wq_a[D, 0:KO] = bq * a10s
    wq_a[D, KO] = b10c

    csth = np.zeros((P, CWH), np.float32)
    csth[:, 0:P] = np.eye(P)
    csth[:, P : 2 * P] = wkvB
    csth[0:DA, 2 * P : 2 * P + KF] = wq_a
    csth = csth.astype(ml_dtypes.bfloat16)
    # merge matrix: sum the 4 column-group partials during the epilogue
    # transpose-matmul; out[q, f] = sum_g out_big[32g + f, q]
    cst32 = np.zeros((P, VA), np.float32)
    for g in range(4):
        for f in range(VA):
            cst32[32 * g + f, f] = 1.0
    cst32 = cst32.astype(ml_dtypes.bfloat16)
    return x, csth, cst32


def _run(x, gamma, beta, W, **spmd_kwargs):
    nc = _get_nc()
    x, csth, cst32 = _host_prep(x, gamma, beta, W)
    in_maps = []
    for c in range(NCORES):
        in_maps.append({
            "x": x,
            "xq": np.ascontiguousarray(x[c * NQ : (c + 1) * NQ]),
            "csth": csth,
            "cst32": cst32,
        })
    res = run_bass_kernel_spmd(
        nc, in_maps, core_ids=list(range(NCORES)), **spmd_kwargs
    )
    out = np.concatenate([res.results[c]["y"] for c in range(NCORES)], axis=0)
    return out, res


def kernel(x, gamma, beta, W):
    out, _ = _run(x, gamma, beta, W)
    return out
